# revision 46
# baseline (speedup 1.0000x reference)
"""Trainium2 Bass kernel for nn_Attention_57166014709861.

8-batch image attention (B=8, C=384, h=8, d=48, HW=1024), data-parallel:
one batch image per NeuronCore, weights broadcast, host-side gather.

v3 (~94us measured; v2 107us, v1 119-129us): the whole kernel is ONE
software pipeline paced by the exp engines (ACT+DVE), which own the
critical path (~74us of psum->sbuf work split across two engines).
Attention runs as a FLAT 33-step pipeline (sT for global slot k + av
for slot k-1) so the exp engines never drain at pair boundaries; PE
warmup matmuls (full-array K=128 x M=128 x N=512 -- partial-array ones
do NOT register as busy to HAM) burn the input-DMA wait so q0/k0 run
at 2.4 GHz; the Tile scheduler re-orders engine queues by its own
priority heuristic, so anything emitted for "later" that an engine
could run early (dep-light dummies, cross-engine multiplies) must be
engine-assigned so a deferred dependency can never head-of-line block
exp casts (this cost 5-7us twice before being rooted out).

  * Skewed emission: attention blocks att(t,yt) start right after pair
    0's q/k land (~15us instead of ~33us); the remaining qkv groups
    (q1-3/k1-3, v) are sprinkled one-per-yt into the attention stream so
    the PE (which has ~50% slack at the exp-bound pace) computes them in
    the gaps and HAM stays warm.
  * PSUM in single-bank [128,512] halves (6 rotating "h" slots shared by
    qkv/v/sT/proj): paired matmuls run CONCURRENTLY on disjoint PE
    sub-arrays (row groups 0/64 for the K<=49 sT, col groups 0/64 for
    the M=64 av) -- tile_position auto-derives from base partitions.
  * Static per-head exp split: even heads (s=0, partitions 0:48) use ACT
    native Exp; odd heads (s=1, partitions 64:113) use DVE. For DVE
    heads the Schraudolph affine map x = A*s + B is folded INTO the sT
    matmul (A rides the host-packed w_q scale, B rides contraction row
    112: q row 112 = 1, k row 112 = B, delivered by a tiny dep-free DMA
    since engines cannot address base partition 112; the q/k copies only
    write partitions 0:112 so the rows survive). DVE then runs a pure
    copy-convert f32->u16: trunc(x) viewed as bf16 bits == bf16(exp(s)),
    zero-mean +-4% err. bf16 rounds B to 16256 -- an integer shift of B
    is an exact constant factor on p that cancels in the normalization.
    Softmax-argument noise does NOT average away, so B must stay
    zero-mean-calibrated and anything coarser than bf16 on the q/k path
    blows the 2e-2 gate (fp8 measured 4.9e-2).
  * Epilogue re-phased to avoid engine FIFO head-of-line blocking: the
    denominator-scatter DMAs fire at the next pair's yt1/yt0, the DVE
    reciprocal only at yt3/yt2 (its dep already landed -> no stall), the
    broadcast DMA at yt4/yt3. Pairs 0/1 normalize on GPSIMD; pair 2 on
    DVE (its broadcast lands after exp is done, and GPSIMD's queue would
    be blocked); pair 3 skips the DMA bounce entirely: 65-lane DVE
    reciprocal straight off av_copy rows 0..64, K=1 ones-matmul
    broadcast on the PE, DVE multiply from PSUM.
  * proj is kt-outer (all three output tiles' kt round together) so each
    round starts the moment its u tile lands; kt3 lands last after the
    final pair's u3.
"""

import sys

if "/opt/trn_rl_repo" not in sys.path:
    sys.path.insert(0, "/opt/trn_rl_repo")

import numpy as np

import concourse.bass as bass
import concourse.mybir as mybir
import concourse.tile as tile
from concourse import bacc
from concourse.bass_utils import run_bass_kernel_spmd

DIM = 384
HEADS = 8
DH = 48
SEQ = 1024
P = 128
NCORES = 8
VP = 64  # packed v cols per head: ones at col 0, zeros 1-15, 48 data at 16-63

F32 = mybir.dt.float32
BF16 = mybir.dt.bfloat16
U16 = mybir.dt.uint16
EXP = mybir.ActivationFunctionType.Exp
IDENT = mybir.ActivationFunctionType.Identity
ADD = mybir.AluOpType.add
MULT = mybir.AluOpType.mult

# Schraudolph exp-as-bf16-bits: trunc(s*A + B) viewed as bf16 ~= exp(s).
# A = 128/ln2; B calibrated for ZERO-MEAN rel err (max 4.2%, rms 1.8%)
# under truncation.
A_EXP = 184.6649652337873
B_EXP = 16249.125

_NC_CACHE = {}


def _emit(tc, nc, x_d, wq0_d, wq12_d, wk0_d, wk12_d, wv_d, wp_d, b_d, brow_d, out_d):
    with (
        tc.tile_pool(name="const", bufs=1) as constp,
        tc.tile_pool(name="weights", bufs=1) as wpool,
        tc.tile_pool(name="data", bufs=1) as data,
        tc.tile_pool(name="ptile", bufs=16) as ppool,
        tc.tile_pool(name="bcpool", bufs=3) as bcpool,
        tc.tile_pool(name="rpool", bufs=3) as rpool,
        tc.tile_pool(name="avcp", bufs=3) as avcp,
        tc.tile_pool(name="opool", bufs=3) as opool,
        tc.tile_pool(name="ps_h", bufs=6, space="PSUM") as ps_h,
        tc.tile_pool(name="ps_av", bufs=1, space="PSUM") as ps_av,
        tc.tile_pool(name="dram", bufs=3, space="DRAM") as drampool,
    ):
        # ---- loads (bf16, 3 queues). Dependency tracking is per-tile
        # (with subtile refinement): x per ko chunk, wq/wk split ko0 vs
        # ko12 so pair 0's q/k matmuls only wait for x+wq0/wk0. x2 rides
        # the scalar queue so all of x lands by ~13us (q0/k0 accumulate
        # all three ko chunks before the first sT can go).
        x_ch = [
            data.tile([P, SEQ], BF16, tag=f"x{ko}", name=f"x{ko}")
            for ko in range(3)
        ]
        wq0_sb = wpool.tile([P, 512], BF16, tag="wq0")
        wq12_sb = wpool.tile([P, 2, 512], BF16, tag="wq12")
        wk0_sb = wpool.tile([P, 512], BF16, tag="wk0")
        wk12_sb = wpool.tile([P, 2, 512], BF16, tag="wk12")
        wv_sb = wpool.tile([P, 3, HEADS * VP], BF16, tag="wv")
        wp_sb = wpool.tile([P, 4, DIM], BF16, tag="wp")
        bias_sb = constp.tile([P, 3], F32, tag="bias")

        q_sb = data.tile([P, 4, SEQ], BF16, tag="q")
        k_sb = data.tile([P, 4, SEQ], BF16, tag="k")
        vT_sb = data.tile([P, 8, HEADS, VP], BF16, tag="vT")

        # memsets BEFORE the gpsimd dma_starts: each doorbell costs
        # ~650ns of gpsimd queue time and the PE warmup needs ones_sb
        # at ~7.3us, not ~11us
        zb_sb = constp.tile([P, 1], F32, tag="zb")
        nc.gpsimd.memset(zb_sb[:], 0.0)
        # rows 0 and 32 are used as K=1 matmul lhsT for the final-pair
        # broadcast (base partition must be 0/32/64/96); full [128,512]
        # so the PE warmup matmuls can run K=128 x M=128 x N=512 (partial
        # -array warmups do NOT register as busy to HAM and the clock
        # stays gated at 1.2 GHz)
        ones_sb = constp.tile([P, 512], BF16, tag="ones")
        nc.gpsimd.memset(ones_sb[:], 1.0)

        nc.sync.dma_start(x_ch[0][:], x_d.ap()[:, 0, :])
        nc.scalar.dma_start(wq0_sb[:], wq0_d.ap())
        nc.gpsimd.dma_start(wk0_sb[:], wk0_d.ap())
        # Schraudolph affine rows (dep-free: q/k copies never write
        # partition 112+): q row 112 = 1, k row 112 = B.
        nc.gpsimd.dma_start(q_sb[112:113, :, :], brow_d.ap()[0:1, :, :])
        nc.gpsimd.dma_start(k_sb[112:113, :, :], brow_d.ap()[1:2, :, :])
        nc.scalar.dma_start(x_ch[1][:], x_d.ap()[:, 1, :])
        nc.sync.dma_start(x_ch[2][:], x_d.ap()[:, 2, :])
        nc.gpsimd.dma_start(wk12_sb[:], wk12_d.ap())
        nc.scalar.dma_start(wq12_sb[:], wq12_d.ap())
        nc.gpsimd.dma_start(wv_sb[:], wv_d.ap())
        nc.gpsimd.dma_start(wp_sb[:], wp_d.ap())
        nc.scalar.dma_start(bias_sb[:], b_d.ap())

        def wq_ko(ko):
            return wq0_sb[:] if ko == 0 else wq12_sb[:, ko - 1, :]

        def wk_ko(ko):
            return wk0_sb[:] if ko == 0 else wk12_sb[:, ko - 1, :]

        # ---- copy engine rotation (GPSIMD cannot read PSUM) ----
        _ncopy = [0]

        def eng_copy(dst, src):
            if _ncopy[0] % 2 == 0:
                nc.vector.tensor_copy(dst, src)
            else:
                nc.scalar.activation(
                    dst, src, IDENT, bias=zb_sb[0 : src.shape[0], :]
                )
            _ncopy[0] += 1

        # ---- qkv work groups (emitted interleaved into the attention
        # stream below). q/k copies write partitions 0:112 only, so the
        # Schraudolph rows survive. ----
        def qk_group(t, which, j):
            dst, wf = (q_sb, wq_ko) if which == "q" else (k_sb, wk_ko)
            ps = ps_h.tile([P, 512], F32, tag="h", name="qk_ps")
            for ko in range(3):
                nc.tensor.matmul(
                    ps[:],
                    lhsT=wf(ko)[:, t * 128 : (t + 1) * 128],
                    rhs=x_ch[ko][:, j * 512 : (j + 1) * 512],
                    start=(ko == 0),
                    stop=(ko == 2),
                )
            eng_copy(dst[0:112, t, j * 512 : (j + 1) * 512], ps[0:112, :])

        def v_group(yt):
            ps = ps_h.tile([P, 512], F32, tag="h", name="v_ps")
            for ko in range(3):
                nc.tensor.matmul(
                    ps[:],
                    lhsT=x_ch[ko][:, yt * 128 : (yt + 1) * 128],
                    rhs=wv_sb[:, ko, :],
                    start=(ko == 0),
                    stop=(ko == 2),
                )
            eng_copy(
                vT_sb[:, yt, :, :],
                ps[:].rearrange("p (h v) -> p h v", h=HEADS),
            )
            # ones column (col 0) for the softmax denominator (psum row 0/64)
            nc.gpsimd.memset(vT_sb[:, yt, :, 0:1], 1.0)

        # ---- attention state ----
        u_sb = [data.tile([P, SEQ], BF16, tag=f"u{i}", name=f"u{i}") for i in range(4)]
        epi = {}  # pair -> dict with av_copy / den_dram / rec_dram

        # Epilogue for pairs 0-2, phase-split so no engine FIFO-blocks on
        # a DMA that hasn't landed: den scatter DMAs early, reciprocal
        # two yts later (dep already met), broadcast next.
        def epi_den(t):
            st = epi[t]
            den_dram = drampool.tile([2 * SEQ], F32, tag="den")
            nc.sync.dma_start(den_dram[0:SEQ], st["avc"][0:1, :])
            nc.sync.dma_start(den_dram[SEQ : 2 * SEQ], st["avc"][64:65, :])
            den_pm = rpool.tile([P, 16], F32, tag="denpm")
            nc.sync.dma_start(den_pm[:], den_dram[:].rearrange("(p f) -> p f", p=P))
            st["den_pm"] = den_pm

        def epi_recip(t):
            st = epi[t]
            rec_pm = rpool.tile([P, 16], BF16, tag="recpm")
            with nc.allow_low_precision(reason="softmax denom reciprocal to bf16"):
                nc.vector.reciprocal(rec_pm[:], st["den_pm"][:])
            rec_dram = drampool.tile([2 * SEQ], BF16, tag="rec")
            nc.sync.dma_start(rec_dram[:], rec_pm[:])
            st["rec_dram"] = rec_dram

        def epi_bcast(t, split=False):
            st = epi[t]
            bc_sb = bcpool.tile([P, SEQ], BF16, tag="bcs")
            rec_r = st["rec_dram"][:].rearrange("(h f) -> h f", h=2)
            if split:
                # the stride-0 expansion runs at ~25GB/s per queue
                # (128 tiny descriptors); split across two queues when
                # the landing time matters (pair 2 gates proj kt2)
                nc.sync.dma_start(
                    bc_sb[0:64, :], rec_r[0:1, None, :].to_broadcast([1, 64, SEQ])
                )
                nc.scalar.dma_start(
                    bc_sb[64:128, :], rec_r[1:2, None, :].to_broadcast([1, 64, SEQ])
                )
            else:
                nc.sync.dma_start(
                    bc_sb[:], rec_r[:, None, :].to_broadcast([2, 64, SEQ])
                )
            # all-SBUF multiply on Pool keeps DVE free for exp
            nc.gpsimd.tensor_tensor(u_sb[t][:], st["avc"][:], bc_sb[:], MULT)

        # ---- one flat pipeline step: sT for global slot k, av for slot
        # k-1. The av round of a pair's LAST chunk shares a step with the
        # NEXT pair's first sT, so the exp engines never drain at pair
        # boundaries. ----
        def sT_part(t, yt):
            st = epi.setdefault(t, {"p": [[[None] * 2 for _ in range(8)] for _ in range(2)]})
            p_half = st["p"]
            for j in range(2):
                for s in range(2):
                    po, ke = (0, 48) if s == 0 else (64, 49)
                    hp = ps_h.tile([P, 512], F32, tag="h", name="sT_h")
                    nc.tensor.matmul(
                        hp[:],
                        lhsT=k_sb[po : po + ke, t, yt * 128 : (yt + 1) * 128],
                        rhs=q_sb[po : po + ke, t, j * 512 : (j + 1) * 512],
                        start=True,
                        stop=True,
                    )
                    if s == 0:
                        pt = ppool.tile([P, 512], BF16, tag="p", name="p_a")
                        nc.scalar.activation(pt[:], hp[:], EXP, bias=zb_sb[:])
                        p_half[s][yt][j] = pt[:]
                    else:
                        pt = ppool.tile([P, 512], U16, tag="p", name="p_d")
                        nc.vector.tensor_copy(pt[:], hp[:])
                        p_half[s][yt][j] = pt[:].bitcast(BF16)

        def av_part(t, r):
            st = epi[t]
            if r == 0:
                st["av"] = ps_av.tile([P, SEQ], F32, tag="av", name=f"av{t}")
            for j in range(2):
                for s in range(2):
                    po = s * 64
                    h = 2 * t + s
                    nc.tensor.matmul(
                        st["av"][po : po + VP, j * 512 : (j + 1) * 512],
                        lhsT=vT_sb[:, r, h, :],
                        rhs=st["p"][s][r][j],
                        start=(r == 0),
                        stop=(r == 7),
                        skip_group_check=True,
                    )
            if r == 7:
                # drain av to SBUF in halves (DVE + ACT), freeing the av
                # psum slot for the next pair
                avc = avcp.tile([P, SEQ], F32, tag="avc", name=f"avc{t}")
                nc.vector.tensor_copy(avc[:, 0:512], st["av"][:, 0:512])
                nc.scalar.activation(
                    avc[:, 512:SEQ], st["av"][:, 512:SEQ], IDENT, bias=zb_sb[:]
                )
                st["avc"] = avc

        # ---- PE warmup: HAM gates the PE clock to 1.2 GHz until it has
        # seen ~3.4us of sustained activity. Burn the input-DMA wait
        # (~7.1-11us) on tiny K=1 matmuls so q0/k0 run at 2.4 GHz. ----
        def pe_warmup(n):
            wm = ps_h.tile([P, 512], F32, tag="h", name="warm_ps")
            for _ in range(n):
                nc.tensor.matmul(
                    wm[:],
                    lhsT=ones_sb[:, 0:128],
                    rhs=ones_sb[:],
                    start=True,
                    stop=True,
                )

        pe_warmup(8)

        # ---- the schedule: att blocks paced by the exp engines, with
        # qkv groups sprinkled into the PE's slack. Pair 0's q/k go
        # first; v groups early in pair 0 (av(t0,1) needs vT(0));
        # q1/k1 mid-pair-0, q2/k2 + q3/k3 across pair 1. ----
        # sT(0,0..3) needs q0 (both halves) + k0 j0 only; k0 j1 rides the
        # first att step so the exp engines start one group earlier
        for which, j in (("q", 0), ("q", 1), ("k", 0)):
            qk_group(0, which, j)
            # one filler warmup after each group: keeps the PE busy
            # streak unbroken across x-chunk arrival gaps so HAM
            # reliably un-throttles regardless of window phase
            pe_warmup(1)

        filler = {
            (0, 0): lambda: (qk_group(0, "k", 1), v_group(0)),
            (0, 1): lambda: (v_group(1), v_group(2)),
            (0, 2): lambda: (v_group(3), qk_group(1, "q", 0)),
            (0, 3): lambda: (v_group(4), qk_group(1, "q", 1)),
            (0, 4): lambda: (v_group(5), qk_group(1, "k", 0)),
            (0, 5): lambda: (v_group(6), qk_group(1, "k", 1)),
            (0, 6): lambda: v_group(7),
            (1, 0): lambda: qk_group(2, "q", 0),
            (1, 1): lambda: qk_group(2, "q", 1),
            (1, 2): lambda: qk_group(2, "k", 0),
            (1, 3): lambda: qk_group(2, "k", 1),
            (1, 4): lambda: qk_group(3, "q", 0),
            (1, 5): lambda: qk_group(3, "q", 1),
            (1, 6): lambda: qk_group(3, "k", 0),
            (1, 7): lambda: qk_group(3, "k", 1),
        }
        # epilogue hooks: (pair being processed, yt) -> action on a
        # PREVIOUS pair. Pair 2's chain runs one yt earlier and
        # normalizes on DVE (its broadcast lands after exp is done).
        hooks = {
            (1, 1): lambda: epi_den(0),
            (1, 5): lambda: epi_recip(0),
            (1, 6): lambda: epi_bcast(0),
            (2, 1): lambda: epi_den(1),
            (2, 5): lambda: epi_recip(1),
            (2, 6): lambda: epi_bcast(1),
            (3, 0): lambda: epi_den(2),
            (3, 3): lambda: (epi_recip(2), epi_bcast(2, split=True)),
        }

        for k in range(33):
            if k < 32:
                t_s, yt_s = divmod(k, 8)
                sT_part(t_s, yt_s)
            if k > 0:
                t_a, r = divmod(k - 1, 8)
                av_part(t_a, r)
            if k < 32:
                f = filler.get((t_s, yt_s))
                if f:
                    f()
                h = hooks.get((t_s, yt_s))
                if h:
                    h()

        # ---- tail: fin(t3) + proj (kt-outer) ----
        # fin: scatter the denominator rows to 128 lanes (reciprocal is
        # ~6 cyc/element on DVE, so narrow-lane forms lose; DMA cannot
        # read PSUM so the scatters source the av_copy), reciprocal in
        # one 253ns op, scatter back to a row pair for the PE
        # ones-matmul broadcast.
        avc3 = epi[3]["avc"]
        # 64-lane x 32-element form: each scatter hop is ONE DMA per row
        # on its own queue with 64 descriptors (~1.2us) instead of 128
        # (~2.3us); the reciprocal still runs wide enough (64 lanes x 32
        # elems at ~6.3 cyc/elem ~= 260ns)
        den_pm3 = rpool.tile([32, 64], F32, tag="denpm")
        nc.sync.dma_start(den_pm3[:, 0:32], avc3[0:1, :])
        nc.scalar.dma_start(den_pm3[:, 32:64], avc3[64:65, :])
        rec_pm3 = rpool.tile([32, 64], BF16, tag="recpm")
        with nc.allow_low_precision(reason="softmax denom reciprocal to bf16"):
            nc.vector.reciprocal(rec_pm3[:], den_pm3[:])
        rec65 = rpool.tile([33, SEQ], BF16, tag="rec65")
        nc.sync.dma_start(rec65[0:1, :], rec_pm3[:, 0:32])
        nc.scalar.dma_start(rec65[32:33, :], rec_pm3[:, 32:64])

        # proj kt rounds 0-2 (kt0/kt1 start while pair 2/3 epilogues run)
        prh = [
            [ps_h.tile([P, 512], F32, tag="h", name=f"pr{ot}{j}") for j in range(2)]
            for ot in range(3)
        ]
        for kt in range(3):
            for ot in range(3):
                for j in range(2):
                    nc.tensor.matmul(
                        prh[ot][j][:],
                        lhsT=wp_sb[:, kt, ot * 128 : (ot + 1) * 128],
                        rhs=u_sb[kt][:, j * 512 : (j + 1) * 512],
                        start=(kt == 0),
                        stop=False,
                        skip_group_check=True,
                    )

        # final-pair broadcast: K=1 ones-matmuls into the freed av slot
        # (rows 0/64 of rec65 -> psum rows 0:64 / 64:128), then the DVE
        # multiply lands u3 just before kt3 needs it.
        bc_ps = ps_av.tile([P, SEQ], F32, tag="av", name="bc_ps")
        for s in range(2):
            sp = 32 * s
            for j in range(2):
                nc.tensor.matmul(
                    bc_ps[s * 64 : (s + 1) * 64, j * 512 : (j + 1) * 512],
                    lhsT=ones_sb[sp : sp + 1, 0:64],
                    rhs=rec65[sp : sp + 1, j * 512 : (j + 1) * 512],
                    start=True,
                    stop=True,
                )
        nc.vector.tensor_tensor(u_sb[3][:], avc3[:], bc_ps[:], MULT)

        for ot in range(3):
            for j in range(2):
                nc.tensor.matmul(
                    prh[ot][j][:],
                    lhsT=wp_sb[:, 3, ot * 128 : (ot + 1) * 128],
                    rhs=u_sb[3][:, j * 512 : (j + 1) * 512],
                    start=False,
                    stop=True,
                    skip_group_check=True,
                )

        oq = [nc.sync, nc.gpsimd, nc.scalar]
        for ot in range(3):
            o_sb = opool.tile([P, SEQ], BF16, tag="o")
            # bias-copy halves spread over ACT/DVE so the three tiles
            # drain in parallel instead of serializing on ACT
            for j in range(2):
                src = prh[ot][j][:]
                dst = o_sb[:, j * 512 : (j + 1) * 512]
                if (ot + j) % 2 == 1:
                    nc.vector.tensor_scalar(
                        dst, src, bias_sb[:, ot : ot + 1], None, ADD
                    )
                else:
                    nc.scalar.activation(
                        dst, src, IDENT, bias=bias_sb[:, ot : ot + 1]
                    )
            oq[ot].dma_start(out_d.ap()[ot * 128 : (ot + 1) * 128, :], o_sb[:])


def build_nc():
    nc = bacc.Bacc("TRN2", target_bir_lowering=False, debug=False, num_devices=NCORES)
    x_d = nc.dram_tensor("x", [P, 3, SEQ], BF16, kind="ExternalInput")
    wq0_d = nc.dram_tensor("wq0", [P, 512], BF16, kind="ExternalInput")
    wq12_d = nc.dram_tensor("wq12", [P, 2, 512], BF16, kind="ExternalInput")
    wk0_d = nc.dram_tensor("wk0", [P, 512], BF16, kind="ExternalInput")
    wk12_d = nc.dram_tensor("wk12", [P, 2, 512], BF16, kind="ExternalInput")
    wv_d = nc.dram_tensor("wv", [P, 3, HEADS * VP], BF16, kind="ExternalInput")
    wp_d = nc.dram_tensor("wp", [P, 4, DIM], BF16, kind="ExternalInput")
    b_d = nc.dram_tensor("bias", [P, 3], F32, kind="ExternalInput")
    brow_d = nc.dram_tensor("brow", [2, 4, SEQ], BF16, kind="ExternalInput")
    out_d = nc.dram_tensor("out", [DIM, SEQ], BF16, kind="ExternalOutput")

    with tile.TileContext(nc) as tc:
        _emit(tc, nc, x_d, wq0_d, wq12_d, wk0_d, wk12_d, wv_d, wp_d, b_d, brow_d, out_d)
    nc.compile()
    return nc


def pack_inputs(x, w_qkv, w_proj, b_proj):
    """Host-side weight packing. Returns per-core input maps."""
    import ml_dtypes

    x = np.asarray(x, np.float32)
    w_qkv = np.asarray(w_qkv, np.float32)
    w_proj = np.asarray(w_proj, np.float32)
    b_proj = np.asarray(b_proj, np.float32)
    scale = DH ** -0.5
    w_q, w_k, w_v = w_qkv[0:DIM], w_qkv[DIM : 2 * DIM], w_qkv[2 * DIM :]

    WQ = np.zeros((DIM, 512), np.float32)
    WK = np.zeros((DIM, 512), np.float32)
    WV = np.zeros((DIM, HEADS * VP), np.float32)
    WP = np.zeros((512, DIM), np.float32)
    for h in range(HEADS):
        col = (h // 2) * 128 + (h % 2) * 64
        # odd heads run the DVE Schraudolph path: fold A into the scale
        qs = scale * (A_EXP if h % 2 == 1 else 1.0)
        WQ[:, col : col + DH] = (w_q[h * DH : (h + 1) * DH] * qs).T
        WK[:, col : col + DH] = w_k[h * DH : (h + 1) * DH].T
        WV[:, h * VP + 16 : h * VP + 16 + DH] = w_v[h * DH : (h + 1) * DH].T
        WP[col + 16 : col + 16 + DH, :] = w_proj[:, h * DH : (h + 1) * DH].T
    BIAS = np.ascontiguousarray(b_proj.reshape(3, P).T)

    def pm(a, chunks):
        # [(chunks*P), f] -> [P, chunks, f] partition-major bf16 pre-layout
        return np.ascontiguousarray(
            a.reshape(chunks, P, a.shape[-1]).transpose(1, 0, 2)
        ).astype(ml_dtypes.bfloat16)

    WQp, WKp, WVp, WPp = pm(WQ, 3), pm(WK, 3), pm(WV, 3), pm(WP, 4)
    WQ0 = np.ascontiguousarray(WQp[:, 0, :])
    WQ12 = np.ascontiguousarray(WQp[:, 1:3, :])
    WK0 = np.ascontiguousarray(WKp[:, 0, :])
    WK12 = np.ascontiguousarray(WKp[:, 1:3, :])
    BROW = np.empty((2, 4, SEQ), np.float32)
    BROW[0] = 1.0
    BROW[1] = B_EXP
    BROW = BROW.astype(ml_dtypes.bfloat16)
    in_maps = []
    for b in range(NCORES):
        in_maps.append(
            {
                "x": pm(x[b].reshape(DIM, SEQ), 3),
                "wq0": WQ0,
                "wq12": WQ12,
                "wk0": WK0,
                "wk12": WK12,
                "wv": WVp,
                "wp": WPp,
                "bias": BIAS,
                "brow": BROW,
            }
        )
    return in_maps


def run(in_maps, trace=False):
    if "nc" not in _NC_CACHE:
        _NC_CACHE["nc"] = build_nc()
    nc = _NC_CACHE["nc"]
    res = run_bass_kernel_spmd(
        nc, in_maps, core_ids=list(range(NCORES)), trace=trace
    )
    out = np.stack(
        [res.results[i]["out"].astype(np.float32) for i in range(NCORES)]
    )
    return out.reshape(NCORES, DIM, 32, 32), res


def kernel(x, w_qkv, w_proj, b_proj):
    out, _ = run(pack_inputs(x, w_qkv, w_proj, b_proj))
    return out
